# revision 1
# baseline (speedup 1.0000x reference)
"""Training-mode BatchNorm2d over x(64,256,56,56) f32 on 8 trn2 NeuronCores.

Sharding: channel-parallel (32 channels per core) instead of the SyncBN-style
batch sharding — each core owns complete per-channel reductions, so no
cross-core collectives are needed at all.

Per core: 4 channel-blocks of 8 channels. A block's data (all 64 batches,
8 channels, 3136 spatial) lives in 4 SBUF tiles of [128p, 3136] where
partition p = b_lo*8 + c (b = b_hi*16 + b_lo). The block stays resident in
SBUF between the stats pass and the normalize pass, so HBM traffic is the
minimal 2x (one read + one write, ~51 MB/core -> ~144us roofline at
358 GB/s per core).

Stats: bn_stats/bn_aggr on VectorE (a single pass yields mean+var ->
sum+sumsq per partition), then reduced across partitions by a tiny PE
matmul against a (1/N)-scaled block-indicator matrix (yielding
per-channel [mean, E[x^2]] on partitions 0..CBLK-1); per-channel
scale/bias are broadcast back to all 128 partitions with a second tiny
matmul. Normalize: x*A + B in-place, alternating between ACT (Identity
activation with per-partition scale/bias APs) and VectorE
(tensor_scalar) so neither engine is the tail. Input DMAs ride the SP
HWDGE ring, output DMAs the ACT HWDGE ring, so reads and writes
overlap on separate queues; all 16 data tiles fit in SBUF at once
(bufs=16), so the load stream never stalls on slot recycling.

Measured: ~139-160 us on hardware (run-to-run variance from HBM-domain
sharing between core pairs); fabric/HBM roofline is ~118-143 us plus
~17 us of fixed Tile preamble/drain overhead.
"""

from contextlib import ExitStack

import numpy as np

import concourse.bass as bass
import concourse.tile as tile
from concourse import bacc, mybir
from concourse.bass_utils import run_bass_kernel_spmd

F32 = mybir.dt.float32

B, C, H, W = 64, 256, 56, 56
HW = H * W  # 3136
N_CORES = 8
C_LOC = C // N_CORES  # 32 channels per core
CBLK = 4  # channels per resident block
N_BLOCKS = C_LOC // CBLK  # blocks per core
BL = 128 // CBLK  # b_lo values packed per partition dim
BH = B // BL  # tiles (b_hi) per block
SUB = 448  # bn_stats subgroup size (3136 = 7*448, <= 512)
NSUB = HW // SUB  # 7
N_PART_ELEMS = BH * HW  # elems per partition per block = 12544
N_TOT = B * HW  # elems per channel = 200704
EPS = 1e-5

_NC_CACHE = {}


def _build_nc(nbufs=16):
    # Bacc (not plain Bass): its finalize() runs generate_event_semaphores,
    # which splits multi-sem waits — TRN2 instructions carry at most one.
    nc = bacc.Bacc()
    x = nc.dram_tensor("x", [N_BLOCKS, BH, 128, HW], F32, kind="ExternalInput")
    y = nc.dram_tensor("y", [N_BLOCKS, BH, 128, HW], F32, kind="ExternalOutput")
    gamma = nc.dram_tensor("gamma", [CBLK, N_BLOCKS], F32, kind="ExternalInput")
    beta = nc.dram_tensor("beta", [CBLK, N_BLOCKS], F32, kind="ExternalInput")
    sel8 = nc.dram_tensor("sel8", [128, CBLK], F32, kind="ExternalInput")
    selT = nc.dram_tensor("selT", [CBLK, 128], F32, kind="ExternalInput")

    with ExitStack() as ctx:
        tc = ctx.enter_context(tile.TileContext(nc))
        xpool = ctx.enter_context(tc.tile_pool(name="xdata", bufs=nbufs))
        spool = ctx.enter_context(tc.tile_pool(name="stats", bufs=4))
        cpool = ctx.enter_context(tc.tile_pool(name="const", bufs=1))
        ppool = ctx.enter_context(tc.tile_pool(name="psum", bufs=2, space="PSUM"))

        sel8_t = cpool.tile([128, CBLK], F32)
        nc.gpsimd.dma_start(out=sel8_t, in_=sel8[:, :])
        selT_t = cpool.tile([CBLK, 128], F32)
        nc.gpsimd.dma_start(out=selT_t, in_=selT[:, :])
        gam_t = cpool.tile([CBLK, N_BLOCKS], F32)
        nc.gpsimd.dma_start(out=gam_t, in_=gamma[:, :])
        bet_t = cpool.tile([CBLK, N_BLOCKS], F32)
        nc.gpsimd.dma_start(out=bet_t, in_=beta[:, :])
        eps_t = cpool.tile([CBLK, 1], F32)
        nc.vector.memset(eps_t, EPS)

        def stats_phase(blk):
            """Loads + bn_stats + per-partition sums + reduce matmul.

            No cross-engine waits land on VectorE here (bn_aggr and the
            conversions only consume VectorE-produced data), so its
            instruction stream never stalls.
            """
            stats = spool.tile([128, BH, NSUB, 6], F32)
            xts = []
            for bh in range(BH):
                xt = xpool.tile([128, HW], F32, tag="x")
                nc.sync.dma_start(out=xt, in_=x[blk, bh, :, :])
                xts.append(xt)
                xv = xt.rearrange("p (s f) -> p s f", f=SUB)
                for j in range(NSUB):
                    nc.vector.bn_stats(out=stats[:, bh, j, :], in_=xv[:, j, :])

            # mean/var per partition over this block's elems
            mv = spool.tile([128, 2], F32)
            nc.vector.bn_aggr(out=mv, in_=stats[:, :, :, :])
            # convert to (sum, sumsq): sum = n*mean, sumsq = n*(var + mean^2)
            m2 = spool.tile([128, 1], F32)
            nc.vector.tensor_mul(m2, mv[:, 0:1], mv[:, 0:1])
            vp = spool.tile([128, 1], F32)
            nc.vector.tensor_add(vp, mv[:, 1:2], m2)
            sums = spool.tile([128, 2], F32)
            nc.vector.tensor_scalar_mul(sums[:, 0:1], mv[:, 0:1], float(N_PART_ELEMS))
            nc.vector.tensor_scalar_mul(sums[:, 1:2], vp, float(N_PART_ELEMS))

            # cross-partition reduce: per-channel [mean, E[x^2]] on
            # partitions 0..CBLK-1 via a tiny PE matmul against the
            # (1/N)-scaled block-indicator matrix.
            tot8 = ppool.tile([CBLK, 2], F32, tag="ps1")
            nc.tensor.matmul(tot8, sel8_t, sums, start=True, stop=True)
            return xts, tot8

        def norm_phase(blk, xts, tot8):
            """Chain tail + normalize + stores. Emitted one block late so
            the PE/ACT round-trips (matmul, sqrt) finish while VectorE is
            streaming the next block's bn_stats — its in-order stream then
            never waits on another engine."""
            me8 = spool.tile([CBLK, 2], F32)
            nc.vector.tensor_copy(me8, tot8)
            m28 = spool.tile([CBLK, 1], F32)
            nc.vector.tensor_mul(m28, me8[:, 0:1], me8[:, 0:1])
            var8 = spool.tile([CBLK, 1], F32)
            nc.vector.tensor_sub(var8, me8[:, 1:2], m28)
            std8 = spool.tile([CBLK, 1], F32)
            nc.scalar.activation(
                std8, var8, mybir.ActivationFunctionType.Sqrt, bias=eps_t
            )
            rstd8 = spool.tile([CBLK, 1], F32)
            nc.vector.reciprocal(rstd8, std8)
            # A = gamma*rstd, B = beta - mean*A
            ab8 = spool.tile([CBLK, 2], F32)
            nc.vector.tensor_mul(ab8[:, 0:1], rstd8, gam_t[:, blk : blk + 1])
            t8 = spool.tile([CBLK, 1], F32)
            nc.vector.tensor_mul(t8, me8[:, 0:1], ab8[:, 0:1])
            nc.vector.tensor_sub(ab8[:, 1:2], bet_t[:, blk : blk + 1], t8)

            # broadcast (A, B) back to all 128 partitions via PE matmul
            ps2 = ppool.tile([128, 2], F32, tag="ps2")
            nc.tensor.matmul(ps2, selT_t, ab8, start=True, stop=True)
            ab = spool.tile([128, 2], F32)
            nc.vector.tensor_copy(ab, ps2)

            for bh in range(BH):
                # split the normalize pass across ACT and VectorE: during
                # the out-only drain phase the fabric needs a normalized
                # tile every ~3.7us, which ACT alone (3us/tile + DMA
                # pushes) cannot sustain — two engines can
                if bh % 2 == 0:
                    nc.scalar.activation(
                        xts[bh],
                        xts[bh],
                        mybir.ActivationFunctionType.Identity,
                        bias=ab[:, 1:2],
                        scale=ab[:, 0:1],
                    )
                else:
                    nc.vector.tensor_scalar(
                        out=xts[bh],
                        in0=xts[bh],
                        scalar1=ab[:, 0:1],
                        scalar2=ab[:, 1:2],
                        op0=mybir.AluOpType.mult,
                        op1=mybir.AluOpType.add,
                    )
                nc.scalar.dma_start(out=y[blk, bh, :, :], in_=xts[bh])

        # One-block-deep software pipeline over the emission order.
        # Block 0 is NOT deferred: at that point VectorE is idle waiting
        # for block 1's loads anyway, so its cross-engine chain stalls are
        # free — and the store stream starts ~8us earlier.
        prev = None
        for blk in range(N_BLOCKS):
            cur = stats_phase(blk)
            if blk == 0:
                norm_phase(blk, *cur)
            else:
                if prev is not None:
                    norm_phase(prev[0], *prev[1])
                prev = (blk, cur)
        if prev is not None:
            norm_phase(prev[0], *prev[1])
    nc.finalize()
    return nc


def get_nc(nbufs=16):
    if nbufs not in _NC_CACHE:
        _NC_CACHE[nbufs] = _build_nc(nbufs)
    return _NC_CACHE[nbufs]


def _sel_matrices():
    # sel8 carries the 1/N so the reduce-matmul yields [mean, E[x^2]]
    sel8 = np.zeros((128, CBLK), dtype=np.float32)
    sel8[np.arange(128), np.arange(128) % CBLK] = 1.0 / N_TOT
    selT = np.zeros((CBLK, 128), dtype=np.float32)
    selT[np.arange(128) % CBLK, np.arange(128)] = 1.0
    return sel8, selT


def pack_inputs(x, gamma, beta):
    """Full inputs -> list of per-core in_maps (device layout)."""
    x = np.asarray(x, dtype=np.float32)
    gamma = np.asarray(gamma, dtype=np.float32)
    beta = np.asarray(beta, dtype=np.float32)
    # [b_hi, b_lo, core, blk, cc, hw] -> [core, blk, b_hi, b_lo, cc, hw]
    xr = np.ascontiguousarray(
        x.reshape(BH, BL, N_CORES, N_BLOCKS, CBLK, HW).transpose(2, 3, 0, 1, 4, 5)
    )
    g = gamma.reshape(N_CORES, N_BLOCKS, CBLK)
    bt = beta.reshape(N_CORES, N_BLOCKS, CBLK)
    sel8, selT = _sel_matrices()
    in_maps = []
    for i in range(N_CORES):
        in_maps.append(
            {
                "x": xr[i].reshape(N_BLOCKS, BH, 128, HW),
                "gamma": np.ascontiguousarray(g[i].T),
                "beta": np.ascontiguousarray(bt[i].T),
                "sel8": sel8,
                "selT": selT,
            }
        )
    return in_maps


def unpack_outputs(per_core_y):
    """List of per-core y (device layout) -> full (64,256,56,56)."""
    ys = np.stack(per_core_y)  # [core, blk, b_hi, 128, hw]
    out = (
        ys.reshape(N_CORES, N_BLOCKS, BH, BL, CBLK, HW)
        .transpose(2, 3, 0, 1, 4, 5)
        .reshape(B, C, H, W)
    )
    return np.ascontiguousarray(out)


def run(inputs, trace=False, nbufs=16):
    """Returns (full_output, BassKernelResults)."""
    nc = get_nc(nbufs)
    in_maps = pack_inputs(inputs["x"], inputs["gamma"], inputs["beta"])
    res = run_bass_kernel_spmd(
        nc, in_maps, list(range(N_CORES)), trace=trace
    )
    out = unpack_outputs([r["y"] for r in res.results])
    return out, res


def kernel(**inputs):
    out, _ = run(inputs)
    return out



# revision 2
# speedup vs baseline: 1.5587x; 1.5587x over previous
"""Training-mode BatchNorm2d over x(64,256,56,56) f32 on 8 trn2 NeuronCores.

Sharding: channel-parallel (32 channels per core) — each core owns complete
per-channel reductions, so no cross-core collectives are needed.

The 2e-2 rel-err budget admits a bf16 HBM data path: the host converts x to
bf16 (max rounding error ~2^-9 of value), the device reads bf16, computes
stats in f32 (bn_stats/bn_aggr accumulate internally in f32), normalizes,
and writes bf16 back; the host converts the output to f32. HBM traffic per
core halves to 12.85 MB read + 12.85 MB write -> ~72us roofline at the
358 GB/s per-core HBM limit, vs ~144us for the f32 path.

Per core: 8 channel-blocks of 4 channels. A block's data (all 64 batches,
4 channels, 3136 spatial) lives in ONE SBUF tile of [128p, 6272] bf16 where
partition p = b_lo*4 + cc (b = b_hi*32 + b_lo, free = b_hi*3136 + hw);
1.6 MB per DMA. All 8 block tiles fit in SBUF at once (12.25 MB), so the
tile stays resident between the stats pass and the normalize pass (minimal
2x HBM traffic) and the load stream never stalls on slot recycling.

Stats: bn_stats/bn_aggr on VectorE (one pass -> per-partition mean/var),
var is converted to E[x^2] in place (2 tiny DVE ops), then reduced across
partitions by a tiny PE matmul against a (1/32)-scaled block-indicator
matrix (yielding per-channel [mean, E[x^2]] on partitions 0..3). The
scalar tail (var, sqrt, A=gamma*rstd, B=beta-mean*A) runs on ScalarE/ACT
(reciprocal on DVE), per-channel (A, B) are broadcast back to all 128
partitions with a second tiny matmul. Normalize: x*A + B in-place on ACT
(Identity activation with per-partition scale/bias APs) for blocks 0-5;
blocks 6-7 normalize on DVE (tensor_scalar, 4x bf16 mode) so the drain
tail is not serialized behind ACT. Input DMAs ride the SP HWDGE ring,
output DMAs the ACT HWDGE ring, so reads and writes overlap on separate
queues.
"""

from contextlib import ExitStack

import ml_dtypes
import numpy as np

import concourse.bass as bass
import concourse.tile as tile
from concourse import bacc, mybir
from concourse.bass_utils import run_bass_kernel_spmd

F32 = mybir.dt.float32
BF16 = mybir.dt.bfloat16
NP_BF16 = np.dtype(ml_dtypes.bfloat16)

B, C, H, W = 64, 256, 56, 56
HW = H * W  # 3136
N_CORES = 8
C_LOC = C // N_CORES  # 32 channels per core
CBLK = 4  # channels per resident block
N_BLOCKS = C_LOC // CBLK  # 8 blocks per core
BL = 128 // CBLK  # 32 b_lo values packed in the partition dim
BH = B // BL  # 2 b_hi groups, merged into the free dim
FREE = BH * HW  # 6272 elems per partition per block
SUB = 448  # bn_stats subgroup size (6272 = 14*448, <= 512)
NSUB = FREE // SUB  # 14
EPS = 1e-5

_NC_CACHE = {}


def _build_nc(nbufs=8):
    # Bacc (not plain Bass): its finalize() runs generate_event_semaphores,
    # which splits multi-sem waits — TRN2 instructions carry at most one.
    nc = bacc.Bacc()
    x = nc.dram_tensor("x", [N_BLOCKS, 128, FREE], BF16, kind="ExternalInput")
    y = nc.dram_tensor("y", [N_BLOCKS, 128, FREE], BF16, kind="ExternalOutput")
    gamma = nc.dram_tensor("gamma", [CBLK, N_BLOCKS], F32, kind="ExternalInput")
    beta = nc.dram_tensor("beta", [CBLK, N_BLOCKS], F32, kind="ExternalInput")
    sel8 = nc.dram_tensor("sel8", [128, CBLK], F32, kind="ExternalInput")
    selT = nc.dram_tensor("selT", [CBLK, 128], F32, kind="ExternalInput")

    AF = mybir.ActivationFunctionType

    with ExitStack() as ctx:
        tc = ctx.enter_context(tile.TileContext(nc))
        xpool = ctx.enter_context(tc.tile_pool(name="xdata", bufs=nbufs))
        spool = ctx.enter_context(tc.tile_pool(name="stats", bufs=4))
        cpool = ctx.enter_context(tc.tile_pool(name="const", bufs=1))
        ppool = ctx.enter_context(tc.tile_pool(name="psum", bufs=2, space="PSUM"))

        sel8_t = cpool.tile([128, CBLK], F32)
        nc.gpsimd.dma_start(out=sel8_t, in_=sel8[:, :])
        selT_t = cpool.tile([CBLK, 128], F32)
        nc.gpsimd.dma_start(out=selT_t, in_=selT[:, :])
        gam_t = cpool.tile([CBLK, N_BLOCKS], F32)
        nc.gpsimd.dma_start(out=gam_t, in_=gamma[:, :])
        bet_t = cpool.tile([CBLK, N_BLOCKS], F32)
        nc.gpsimd.dma_start(out=bet_t, in_=beta[:, :])
        eps_t = cpool.tile([CBLK, 1], F32)
        nc.vector.memset(eps_t, EPS)

        def stats_phase(blk):
            """Load + bn_stats + per-partition [mean, E[x^2]] + reduce matmul.

            No cross-engine waits land on VectorE here, so its instruction
            stream never stalls.
            """
            xt = xpool.tile([128, FREE], BF16, tag="x")
            nc.sync.dma_start(out=xt, in_=x[blk, :, :])
            stats = spool.tile([128, NSUB, 6], F32)
            xv = xt.rearrange("p (s f) -> p s f", f=SUB)
            for j in range(NSUB):
                nc.vector.bn_stats(out=stats[:, j, :], in_=xv[:, j, :])

            # per-partition mean/var over this block's elems
            mv = spool.tile([128, 2], F32)
            nc.vector.bn_aggr(out=mv, in_=stats)
            # convert var -> E[x^2] in place: mv[:,1] += mean^2
            m2 = spool.tile([128, 1], F32)
            nc.vector.tensor_mul(m2, mv[:, 0:1], mv[:, 0:1])
            nc.vector.tensor_add(mv[:, 1:2], mv[:, 1:2], m2)

            # cross-partition reduce: per-channel [mean, E[x^2]] on
            # partitions 0..CBLK-1 via a tiny PE matmul against the
            # (1/32)-scaled block-indicator matrix.
            tot = ppool.tile([CBLK, 2], F32, tag="ps1")
            nc.tensor.matmul(tot, sel8_t, mv, start=True, stop=True)
            return xt, tot

        def norm_phase(blk, xt, tot, use_dve):
            """Scalar tail + normalize + store. Emitted one block late so the
            PE/ACT round-trips finish while VectorE is streaming the next
            block's bn_stats — its in-order stream then never waits on
            another engine."""
            me8 = spool.tile([CBLK, 2], F32)
            nc.scalar.activation(me8, tot, AF.Copy)
            m28 = spool.tile([CBLK, 1], F32)
            nc.scalar.activation(m28, me8[:, 0:1], AF.Square)
            var8 = spool.tile([CBLK, 1], F32)
            # E[x^2] - mean^2
            nc.scalar.activation(var8, m28, AF.Identity, scale=-1.0, bias=me8[:, 1:2])
            std8 = spool.tile([CBLK, 1], F32)
            nc.scalar.activation(std8, var8, AF.Sqrt, bias=eps_t)
            rstd8 = spool.tile([CBLK, 1], F32)
            nc.vector.reciprocal(rstd8, std8)
            # A = gamma*rstd, B = beta - mean*A
            ab8 = spool.tile([CBLK, 2], F32)
            nc.scalar.activation(ab8[:, 0:1], rstd8, AF.Copy, scale=gam_t[:, blk : blk + 1])
            t8 = spool.tile([CBLK, 1], F32)
            nc.scalar.activation(t8, me8[:, 0:1], AF.Copy, scale=ab8[:, 0:1])
            nc.scalar.activation(
                ab8[:, 1:2], t8, AF.Identity, scale=-1.0, bias=bet_t[:, blk : blk + 1]
            )

            # broadcast (A, B) back to all 128 partitions via PE matmul
            ps2 = ppool.tile([128, 2], F32, tag="ps2")
            nc.tensor.matmul(ps2, selT_t, ab8, start=True, stop=True)
            ab = spool.tile([128, 2], F32)
            nc.scalar.activation(ab, ps2, AF.Copy)

            if use_dve:
                nc.vector.tensor_scalar(
                    out=xt,
                    in0=xt,
                    scalar1=ab[:, 0:1],
                    scalar2=ab[:, 1:2],
                    op0=mybir.AluOpType.mult,
                    op1=mybir.AluOpType.add,
                )
            else:
                nc.scalar.activation(
                    xt, xt, AF.Identity, bias=ab[:, 1:2], scale=ab[:, 0:1]
                )
            nc.scalar.dma_start(out=y[blk, :, :], in_=xt)

        # One-block-deep software pipeline over the emission order.
        # Block 0 is NOT deferred: at that point VectorE is idle waiting
        # for block 1's load anyway, so its cross-engine chain stalls are
        # free — and the store stream starts earlier.
        prev = None
        for blk in range(N_BLOCKS):
            cur = stats_phase(blk)
            if blk == 0:
                norm_phase(blk, *cur, use_dve=False)
            else:
                if prev is not None:
                    pblk = prev[0]
                    norm_phase(pblk, *prev[1], use_dve=(pblk >= N_BLOCKS - 2))
                prev = (blk, cur)
        if prev is not None:
            pblk = prev[0]
            norm_phase(pblk, *prev[1], use_dve=(pblk >= N_BLOCKS - 2))
    nc.finalize()
    return nc


def get_nc(nbufs=8):
    if nbufs not in _NC_CACHE:
        _NC_CACHE[nbufs] = _build_nc(nbufs)
    return _NC_CACHE[nbufs]


def _sel_matrices():
    # sel8 carries the 1/BL so the reduce-matmul on per-partition
    # [mean, E[x^2]] yields the per-channel values directly
    sel8 = np.zeros((128, CBLK), dtype=np.float32)
    sel8[np.arange(128), np.arange(128) % CBLK] = 1.0 / BL
    selT = np.zeros((CBLK, 128), dtype=np.float32)
    selT[np.arange(128) % CBLK, np.arange(128)] = 1.0
    return sel8, selT


def pack_inputs(x, gamma, beta):
    """Full f32 inputs -> list of per-core in_maps (bf16 device layout)."""
    x16 = np.asarray(x, dtype=np.float32).astype(NP_BF16)
    gamma = np.asarray(gamma, dtype=np.float32)
    beta = np.asarray(beta, dtype=np.float32)
    # [b_hi, b_lo, core, blk, cc, hw] -> [core, blk, b_lo, cc, b_hi, hw]
    xr = np.ascontiguousarray(
        x16.reshape(BH, BL, N_CORES, N_BLOCKS, CBLK, HW).transpose(2, 3, 1, 4, 0, 5)
    )
    g = gamma.reshape(N_CORES, N_BLOCKS, CBLK)
    bt = beta.reshape(N_CORES, N_BLOCKS, CBLK)
    sel8, selT = _sel_matrices()
    in_maps = []
    for i in range(N_CORES):
        in_maps.append(
            {
                "x": xr[i].reshape(N_BLOCKS, 128, FREE),
                "gamma": np.ascontiguousarray(g[i].T),
                "beta": np.ascontiguousarray(bt[i].T),
                "sel8": sel8,
                "selT": selT,
            }
        )
    return in_maps


def unpack_outputs(per_core_y):
    """List of per-core y (bf16 device layout) -> full f32 (64,256,56,56)."""
    ys = np.stack(per_core_y)  # [core, blk, 128, free] bf16
    out = (
        ys.reshape(N_CORES, N_BLOCKS, BL, CBLK, BH, HW)
        .transpose(4, 2, 0, 1, 3, 5)
        .astype(np.float32)
        .reshape(B, C, H, W)
    )
    return out


def run(inputs, trace=False, nbufs=8):
    """Returns (full_output, BassKernelResults)."""
    nc = get_nc(nbufs)
    in_maps = pack_inputs(inputs["x"], inputs["gamma"], inputs["beta"])
    res = run_bass_kernel_spmd(nc, in_maps, list(range(N_CORES)), trace=trace)
    out = unpack_outputs([r["y"] for r in res.results])
    return out, res


def kernel(**inputs):
    out, _ = run(inputs)
    return out


# revision 4
# speedup vs baseline: 1.7665x; 1.1333x over previous
"""Training-mode BatchNorm2d over x(64,256,56,56) f32 on 8 trn2 NeuronCores.

Sharding: channel-parallel (32 channels per core) — each core owns complete
per-channel reductions, so no cross-core collectives are needed.

The 2e-2 rel-err budget admits a bf16 HBM data path: the host converts x to
bf16 (max rounding error ~2^-9 of value), the device reads bf16, computes
stats in f32, normalizes, and writes bf16 back; the host converts the output
to f32. HBM traffic per core halves to 12.85 MB read + 12.85 MB write,
~62us at the measured ~26 GB/s per-DMA-engine rate — the bottleneck this
kernel is built around.

Layout: per core 8 channel-blocks of 4 channels; each block is two
half-tiles [128p, 3136] bf16 (partition p = b_lo*4 + cc, half = b_hi), so
16 loads + 16 stores of 800KB. All 16 halves stay resident in SBUF (12.25
MB) between the stats pass and the normalize pass (minimal 2x HBM traffic).

Stats are engineered so no single engine exceeds the DMA floor:
 - per-channel sum on the (otherwise idle) TensorEngine: 7 matmuls per
   half of x-chunks [128, 448] (moving, bf16) against a (1/32)-scaled
   channel-indicator (stationary, bf16; 1/32 is exact in bf16),
   PSUM-accumulated into [4, 448], then one DVE reduce_sum -> Sum(x)/32.
 - per-partition sum-of-squares on ScalarE: Square activation with
   accum_out (f32), one per half (~2.8us), reduced per-channel by a tiny
   f32 matmul against the same indicator.
 - scalar tail (var, rsqrt, A=gamma*rstd, B=beta-mean*A) on DVE (its small
   ops are ~3x cheaper than ACT's); Sqrt on ACT (DVE has none); per-channel
   (A, B) broadcast to 128 partitions by a tiny PE matmul.
 - normalize x*A + B in place on DVE (tensor_scalar hits 4x bf16 mode,
   ~0.95us per half).

Input DMAs ride the SP HWDGE ring (no waits ever land there, so all 16
loads stream back-to-back at full rate); output DMAs ride the GpSimd SWDGE
ring so their wait-for-normalize semaphores never block the ACT or SP
instruction streams.
"""

from contextlib import ExitStack

import ml_dtypes
import numpy as np

import concourse.bass as bass
import concourse.tile as tile
from concourse import bacc, mybir
from concourse.bass_utils import run_bass_kernel_spmd

F32 = mybir.dt.float32
BF16 = mybir.dt.bfloat16
NP_BF16 = np.dtype(ml_dtypes.bfloat16)

B, C, H, W = 64, 256, 56, 56
HW = H * W  # 3136
N_CORES = 8
C_LOC = C // N_CORES  # 32 channels per core
CBLK = 4  # channels per block
N_BLOCKS = C_LOC // CBLK  # 8 blocks per core
BL = 128 // CBLK  # 32 b_lo values packed in the partition dim
BH = B // BL  # 2 half-tiles per block (b_hi)
N_HALF = N_BLOCKS * BH  # 16 half-tiles per core
SUB = 448  # PE sum-matmul chunk width (3136 = 7*448, <= 512 PSUM cols)
NSUB = HW // SUB  # 7
N_TOT = B * HW  # elems per channel = 200704
N_PART = BH * HW  # elems per partition per block = 6272
EPS = 1e-5

_NC_CACHE = {}


def _build_nc(nbufs=16):
    # Bacc (not plain Bass): its finalize() runs generate_event_semaphores,
    # which splits multi-sem waits — TRN2 instructions carry at most one.
    nc = bacc.Bacc()
    x = nc.dram_tensor("x", [N_HALF, 128, HW], BF16, kind="ExternalInput")
    y = nc.dram_tensor("y", [N_HALF, 128, HW], BF16, kind="ExternalOutput")
    gamma = nc.dram_tensor("gamma", [CBLK, N_BLOCKS], F32, kind="ExternalInput")
    beta = nc.dram_tensor("beta", [CBLK, N_BLOCKS], F32, kind="ExternalInput")
    sel8b = nc.dram_tensor("sel8b", [128, CBLK], BF16, kind="ExternalInput")
    sel8f = nc.dram_tensor("sel8f", [128, CBLK], F32, kind="ExternalInput")
    selT = nc.dram_tensor("selT", [CBLK, 128], F32, kind="ExternalInput")

    AF = mybir.ActivationFunctionType
    OP = mybir.AluOpType

    with ExitStack() as ctx:
        tc = ctx.enter_context(tile.TileContext(nc))
        xpool = ctx.enter_context(tc.tile_pool(name="xdata", bufs=nbufs))
        qpool = ctx.enter_context(tc.tile_pool(name="sqscr", bufs=2))
        spool = ctx.enter_context(tc.tile_pool(name="stats", bufs=4))
        cpool = ctx.enter_context(tc.tile_pool(name="const", bufs=1))
        ppool = ctx.enter_context(tc.tile_pool(name="psum", bufs=2, space="PSUM"))

        sel8b_t = cpool.tile([128, CBLK], BF16)
        nc.gpsimd.dma_start(out=sel8b_t, in_=sel8b[:, :])
        sel8f_t = cpool.tile([128, CBLK], F32)
        nc.gpsimd.dma_start(out=sel8f_t, in_=sel8f[:, :])
        selT_t = cpool.tile([CBLK, 128], F32)
        nc.gpsimd.dma_start(out=selT_t, in_=selT[:, :])
        gam_t = cpool.tile([CBLK, N_BLOCKS], F32)
        nc.gpsimd.dma_start(out=gam_t, in_=gamma[:, :])
        bet_t = cpool.tile([CBLK, N_BLOCKS], F32)
        nc.gpsimd.dma_start(out=bet_t, in_=beta[:, :])
        eps_t = cpool.tile([CBLK, 1], F32)
        nc.vector.memset(eps_t, EPS)

        for blk in range(N_BLOCKS):
            xts = []
            sq_pp = spool.tile([128, BH], F32)
            psum_s = ppool.tile([CBLK, SUB], F32, tag="ps")
            for j in range(BH):
                h = blk * BH + j
                xt = xpool.tile([128, HW], BF16, tag="x")
                nc.sync.dma_start(out=xt, in_=x[h, :, :])
                xts.append(xt)
                # per-partition sum(x^2) via ACT accumulate; the squared
                # main output goes to a recycled scratch tile
                scr = qpool.tile([128, HW], BF16, tag="scr")
                nc.scalar.activation(
                    scr, xt, AF.Square, accum_out=sq_pp[:, j : j + 1]
                )
                # per-channel sum(x)/32 via PE, accumulated over all 14
                # chunks of the block into one [4, 448] PSUM region
                xv = xt.rearrange("p (s f) -> p s f", f=SUB)
                for s in range(NSUB):
                    nc.tensor.matmul(
                        psum_s,
                        sel8b_t,
                        xv[:, s, :],
                        start=(j == 0 and s == 0),
                        stop=(j == BH - 1 and s == NSUB - 1),
                    )

            # per-channel sum(x^2)/32: fold the two halves' per-partition
            # accumulators first (SBUF), then one tiny reduce matmul
            spp = spool.tile([128, 1], F32)
            nc.vector.tensor_add(spp, sq_pp[:, 0:1], sq_pp[:, 1:2])
            mm2 = ppool.tile([CBLK, 1], F32, tag="pq")
            nc.tensor.matmul(mm2, sel8f_t, spp, start=True, stop=True)

            # fold: mean = (sum/32)/6272, E[x^2] likewise; var = E - mean^2
            s4 = spool.tile([CBLK, 1], F32)
            nc.vector.reduce_sum(s4, psum_s, axis=mybir.AxisListType.X)
            mean = spool.tile([CBLK, 1], F32)
            nc.vector.tensor_scalar_mul(mean, s4, 1.0 / N_PART)
            ex2 = spool.tile([CBLK, 1], F32)
            nc.vector.tensor_copy(ex2, mm2)
            m2 = spool.tile([CBLK, 1], F32)
            nc.vector.tensor_mul(m2, mean, mean)
            var = spool.tile([CBLK, 1], F32)
            nc.vector.tensor_scalar(
                out=var,
                in0=ex2,
                scalar1=1.0 / N_PART,
                scalar2=m2,
                op0=OP.mult,
                op1=OP.subtract,
            )
            std = spool.tile([CBLK, 1], F32)
            nc.scalar.activation(std, var, AF.Sqrt, bias=eps_t)
            rstd = spool.tile([CBLK, 1], F32)
            nc.vector.reciprocal(rstd, std)
            # A = gamma*rstd, B = beta - mean*A
            ab8 = spool.tile([CBLK, 2], F32)
            nc.vector.tensor_mul(ab8[:, 0:1], rstd, gam_t[:, blk : blk + 1])
            t4 = spool.tile([CBLK, 1], F32)
            nc.vector.tensor_mul(t4, mean, ab8[:, 0:1])
            nc.vector.tensor_sub(ab8[:, 1:2], bet_t[:, blk : blk + 1], t4)

            # broadcast (A, B) to all 128 partitions via PE matmul
            ps2 = ppool.tile([128, 2], F32, tag="pb")
            nc.tensor.matmul(ps2, selT_t, ab8, start=True, stop=True)
            ab = spool.tile([128, 2], F32)
            nc.vector.tensor_copy(ab, ps2)

            for j in range(BH):
                h = blk * BH + j
                nc.vector.tensor_scalar(
                    out=xts[j],
                    in0=xts[j],
                    scalar1=ab[:, 0:1],
                    scalar2=ab[:, 1:2],
                    op0=OP.mult,
                    op1=OP.add,
                )
                nc.gpsimd.dma_start(out=y[h, :, :], in_=xts[j])
    nc.finalize()
    return nc


def get_nc(nbufs=16):
    if nbufs not in _NC_CACHE:
        _NC_CACHE[nbufs] = _build_nc(nbufs)
    return _NC_CACHE[nbufs]


def _sel_matrices():
    # the 1/32 channel-indicator: reduce-matmuls on per-partition values
    # yield (sum over the channel's 32 partitions)/32
    sel = np.zeros((128, CBLK), dtype=np.float32)
    sel[np.arange(128), np.arange(128) % CBLK] = 1.0 / BL
    selT = np.zeros((CBLK, 128), dtype=np.float32)
    selT[np.arange(128) % CBLK, np.arange(128)] = 1.0
    return sel, selT


def pack_inputs(x, gamma, beta):
    """Full f32 inputs -> list of per-core in_maps (bf16 device layout)."""
    x16 = np.asarray(x, dtype=np.float32).astype(NP_BF16)
    gamma = np.asarray(gamma, dtype=np.float32)
    beta = np.asarray(beta, dtype=np.float32)
    # [b_hi, b_lo, core, blk, cc, hw] -> [core, blk, b_hi, b_lo, cc, hw]
    xr = np.ascontiguousarray(
        x16.reshape(BH, BL, N_CORES, N_BLOCKS, CBLK, HW).transpose(2, 3, 0, 1, 4, 5)
    )
    g = gamma.reshape(N_CORES, N_BLOCKS, CBLK)
    bt = beta.reshape(N_CORES, N_BLOCKS, CBLK)
    sel, selT = _sel_matrices()
    sel8b = sel.astype(NP_BF16)  # 1/32 is exact in bf16
    in_maps = []
    for i in range(N_CORES):
        in_maps.append(
            {
                "x": xr[i].reshape(N_HALF, 128, HW),
                "gamma": np.ascontiguousarray(g[i].T),
                "beta": np.ascontiguousarray(bt[i].T),
                "sel8b": sel8b,
                "sel8f": sel,
                "selT": selT,
            }
        )
    return in_maps


def unpack_outputs(per_core_y):
    """List of per-core y (bf16 device layout) -> full f32 (64,256,56,56)."""
    ys = np.stack(per_core_y)  # [core, blk*b_hi, 128, hw] bf16
    out = (
        ys.reshape(N_CORES, N_BLOCKS, BH, BL, CBLK, HW)
        .transpose(2, 3, 0, 1, 4, 5)
        .astype(np.float32)
        .reshape(B, C, H, W)
    )
    return out


def run(inputs, trace=False, nbufs=16):
    """Returns (full_output, BassKernelResults)."""
    nc = get_nc(nbufs)
    in_maps = pack_inputs(inputs["x"], inputs["gamma"], inputs["beta"])
    res = run_bass_kernel_spmd(nc, in_maps, list(range(N_CORES)), trace=trace)
    out = unpack_outputs([r["y"] for r in res.results])
    return out, res


def kernel(**inputs):
    out, _ = run(inputs)
    return out


# revision 5
# speedup vs baseline: 1.8677x; 1.0573x over previous
"""Training-mode BatchNorm2d over x(64,256,56,56) f32 on 8 trn2 NeuronCores.

Sharding: channel-parallel (32 channels per core) — each core owns complete
per-channel reductions, so no cross-core collectives are needed.

The 2e-2 rel-err budget admits a bf16 HBM data path: the host converts x to
bf16 (max rounding error ~2^-9 of value), the device reads bf16, computes
stats in f32, normalizes, and writes bf16 back; the host converts the output
to f32. HBM traffic per core halves to 12.85 MB read + 12.85 MB write,
~63us at the measured ~410 GB/s per-core aggregate DMA rate — the floor
this kernel is built around.

Layout: per core 8 channel-blocks of 4 channels; each block is two
half-tiles [128p, 3136] bf16 (partition p = b_lo*4 + cc, half = b_hi), so
16 loads + 16 stores of 800KB. All 16 halves stay resident in SBUF (12.25
MB) between the stats pass and the normalize pass (minimal 2x HBM traffic).

Per block the two streaming moments (sum, sum of squares) are split so no
engine exceeds the ~7.75us/block DMA pace:
 - half 0: DVE bn_stats (one 1x pass yields per-partition mean AND var,
   ~3.7us); var is converted to E[x^2] in place.
 - half 1: ScalarE Square activation with accum_out -> per-partition
   sum(x^2) (~3.4us); per-channel sum(x) on the TensorEngine: 7 matmuls
   of x-chunks [128, 448] (moving, bf16) against a (1/32)-scaled
   channel-indicator (stationary, bf16; 1/32 is exact in bf16),
   PSUM-accumulated into [4, 448] and folded by one DVE reduce_sum.
 - both halves' per-partition stats are packed into one [128, 3] tile and
   reduced per-channel by a single tiny f32 matmul.
 - scalar tail (var, rsqrt, A=gamma*rstd, B=beta-mean*A) on DVE (its small
   ops are ~3x cheaper than ACT's); Sqrt on ACT (DVE has none); (A, B)
   broadcast to 128 partitions by a tiny PE matmul.
 - normalize x*A + B in place: half 0 on ACT (Identity with per-partition
   scale/bias APs, ~2.8us), half 1 on DVE (tensor_scalar, 4x bf16 mode,
   ~1.25us).

The tail of block k is emitted one block late (after block k+1's stats),
so the cross-engine chain latency hides under the next block's streaming
work. Input DMAs ride the SP HWDGE ring (no waits ever land there, so all
16 loads stream back-to-back); output DMAs ride the GpSimd SWDGE ring so
their wait-for-normalize semaphores never block the ACT or SP streams.
"""

from contextlib import ExitStack

import ml_dtypes
import numpy as np

import concourse.bass as bass
import concourse.tile as tile
from concourse import bacc, mybir
from concourse.bass_utils import run_bass_kernel_spmd

F32 = mybir.dt.float32
BF16 = mybir.dt.bfloat16
NP_BF16 = np.dtype(ml_dtypes.bfloat16)

B, C, H, W = 64, 256, 56, 56
HW = H * W  # 3136
N_CORES = 8
C_LOC = C // N_CORES  # 32 channels per core
CBLK = 4  # channels per block
N_BLOCKS = C_LOC // CBLK  # 8 blocks per core
BL = 128 // CBLK  # 32 b_lo values packed in the partition dim
BH = B // BL  # 2 half-tiles per block (b_hi)
N_HALF = N_BLOCKS * BH  # 16 half-tiles per core
SUB = 448  # bn_stats subgroup / PE chunk width (3136 = 7*448, <= 512)
NSUB = HW // SUB  # 7
EPS = 1e-5

_NC_CACHE = {}


def _build_nc(nbufs=16):
    # Bacc (not plain Bass): its finalize() runs generate_event_semaphores,
    # which splits multi-sem waits — TRN2 instructions carry at most one.
    nc = bacc.Bacc()
    x = nc.dram_tensor("x", [N_HALF, 128, HW], BF16, kind="ExternalInput")
    y = nc.dram_tensor("y", [N_HALF, 128, HW], BF16, kind="ExternalOutput")
    gamma = nc.dram_tensor("gamma", [CBLK, N_BLOCKS], F32, kind="ExternalInput")
    beta = nc.dram_tensor("beta", [CBLK, N_BLOCKS], F32, kind="ExternalInput")
    sel8b = nc.dram_tensor("sel8b", [128, CBLK], BF16, kind="ExternalInput")
    sel8f = nc.dram_tensor("sel8f", [128, CBLK], F32, kind="ExternalInput")
    selT = nc.dram_tensor("selT", [CBLK, 128], F32, kind="ExternalInput")

    AF = mybir.ActivationFunctionType
    OP = mybir.AluOpType

    with ExitStack() as ctx:
        tc = ctx.enter_context(tile.TileContext(nc))
        xpool = ctx.enter_context(tc.tile_pool(name="xdata", bufs=nbufs))
        qpool = ctx.enter_context(tc.tile_pool(name="sqscr", bufs=2))
        spool = ctx.enter_context(tc.tile_pool(name="stats", bufs=4))
        cpool = ctx.enter_context(tc.tile_pool(name="const", bufs=1))
        ppool = ctx.enter_context(tc.tile_pool(name="psum", bufs=2, space="PSUM"))

        sel8b_t = cpool.tile([128, CBLK], BF16)
        nc.gpsimd.dma_start(out=sel8b_t, in_=sel8b[:, :])
        sel8f_t = cpool.tile([128, CBLK], F32)
        nc.gpsimd.dma_start(out=sel8f_t, in_=sel8f[:, :])
        selT_t = cpool.tile([CBLK, 128], F32)
        nc.gpsimd.dma_start(out=selT_t, in_=selT[:, :])
        gam_t = cpool.tile([CBLK, N_BLOCKS], F32)
        nc.gpsimd.dma_start(out=gam_t, in_=gamma[:, :])
        bet_t = cpool.tile([CBLK, N_BLOCKS], F32)
        nc.gpsimd.dma_start(out=bet_t, in_=beta[:, :])
        eps_t = cpool.tile([CBLK, 1], F32)
        nc.vector.memset(eps_t, EPS)

        def stats_phase(blk):
            # half 0: DVE bn_stats -> per-partition [mean, var] of h0
            xt0 = xpool.tile([128, HW], BF16, tag="x")
            nc.sync.dma_start(out=xt0, in_=x[blk * BH, :, :])
            pack = spool.tile([128, 3], F32)
            stats = spool.tile([128, NSUB, 6], F32)
            xv0 = xt0.rearrange("p (s f) -> p s f", f=SUB)
            for s in range(NSUB):
                nc.vector.bn_stats(out=stats[:, s, :], in_=xv0[:, s, :])

            # half 1: ACT sum(x^2) into pack[:,2]; PE per-channel sum(x)/32
            xt1 = xpool.tile([128, HW], BF16, tag="x")
            nc.sync.dma_start(out=xt1, in_=x[blk * BH + 1, :, :])
            scr = qpool.tile([128, HW], BF16, tag="scr")
            nc.scalar.activation(scr, xt1, AF.Square, accum_out=pack[:, 2:3])
            psum_s = ppool.tile([CBLK, SUB], F32, tag="ps")
            xv1 = xt1.rearrange("p (s f) -> p s f", f=SUB)
            for s in range(NSUB):
                nc.tensor.matmul(
                    psum_s,
                    sel8b_t,
                    xv1[:, s, :],
                    start=(s == 0),
                    stop=(s == NSUB - 1),
                )

            # DVE: finish per-partition stats (var -> E[x^2] in place)
            nc.vector.bn_aggr(out=pack[:, 0:2], in_=stats)
            m2 = spool.tile([128, 1], F32)
            nc.vector.tensor_mul(m2, pack[:, 0:1], pack[:, 0:1])
            nc.vector.tensor_add(pack[:, 1:2], pack[:, 1:2], m2)

            # PE: per-channel [mean_h0, E2_h0, sumsq_h1] / 32
            pq = ppool.tile([CBLK, 3], F32, tag="pq")
            nc.tensor.matmul(pq, sel8f_t, pack, start=True, stop=True)
            return xt0, xt1, psum_s, pq

        def norm_phase(blk, xt0, xt1, psum_s, pq):
            """Fold + scalar tail + normalize + stores. Emitted one block
            late so the cross-engine round-trips hide under the next
            block's streaming work."""
            # fold PE sums: s4 = sum(x_h1)/32 per channel
            s4 = spool.tile([CBLK, 1], F32)
            nc.vector.reduce_sum(s4, psum_s, axis=mybir.AxisListType.X)
            mh1 = spool.tile([CBLK, 1], F32)
            nc.vector.tensor_scalar_mul(mh1, s4, 1.0 / HW)
            # mean = (mean_h0 + mean_h1)/2
            mean = spool.tile([CBLK, 1], F32)
            nc.vector.tensor_scalar(
                out=mean, in0=pq[:, 0:1], scalar1=mh1, scalar2=0.5,
                op0=OP.add, op1=OP.mult,
            )
            # E[x^2] = (E2_h0 + sumsq_h1/3136)/2
            e1 = spool.tile([CBLK, 1], F32)
            nc.vector.tensor_scalar_mul(e1, pq[:, 2:3], 1.0 / HW)
            ex2 = spool.tile([CBLK, 1], F32)
            nc.vector.tensor_scalar(
                out=ex2, in0=pq[:, 1:2], scalar1=e1, scalar2=0.5,
                op0=OP.add, op1=OP.mult,
            )
            m2b = spool.tile([CBLK, 1], F32)
            nc.vector.tensor_mul(m2b, mean, mean)
            var = spool.tile([CBLK, 1], F32)
            nc.vector.tensor_sub(var, ex2, m2b)
            std = spool.tile([CBLK, 1], F32)
            nc.scalar.activation(std, var, AF.Sqrt, bias=eps_t)
            rstd = spool.tile([CBLK, 1], F32)
            nc.vector.reciprocal(rstd, std)
            # A = gamma*rstd, B = beta - mean*A
            ab8 = spool.tile([CBLK, 2], F32)
            nc.vector.tensor_mul(ab8[:, 0:1], rstd, gam_t[:, blk : blk + 1])
            t4 = spool.tile([CBLK, 1], F32)
            nc.vector.tensor_mul(t4, mean, ab8[:, 0:1])
            nc.vector.tensor_sub(ab8[:, 1:2], bet_t[:, blk : blk + 1], t4)

            # broadcast (A, B) to all 128 partitions via PE matmul
            ps2 = ppool.tile([128, 2], F32, tag="pb")
            nc.tensor.matmul(ps2, selT_t, ab8, start=True, stop=True)
            ab = spool.tile([128, 2], F32)
            nc.vector.tensor_copy(ab, ps2)

            # normalize: half 0 on ACT, half 1 on DVE; stores on SWDGE
            nc.scalar.activation(
                xt0, xt0, AF.Identity, bias=ab[:, 1:2], scale=ab[:, 0:1]
            )
            nc.gpsimd.dma_start(out=y[blk * BH, :, :], in_=xt0)
            nc.vector.tensor_scalar(
                out=xt1, in0=xt1, scalar1=ab[:, 0:1], scalar2=ab[:, 1:2],
                op0=OP.mult, op1=OP.add,
            )
            nc.gpsimd.dma_start(out=y[blk * BH + 1, :, :], in_=xt1)

        # One-block-deep software pipeline over the emission order.
        # Block 0 is NOT deferred: at that point the engines are idle
        # waiting for block 1's loads anyway, so its cross-engine chain
        # stalls are free — and the store stream starts earlier.
        prev = None
        for blk in range(N_BLOCKS):
            cur = stats_phase(blk)
            if blk == 0:
                norm_phase(blk, *cur)
            else:
                if prev is not None:
                    norm_phase(prev[0], *prev[1])
                prev = (blk, cur)
        if prev is not None:
            norm_phase(prev[0], *prev[1])
    nc.finalize()
    return nc


def get_nc(nbufs=16):
    if nbufs not in _NC_CACHE:
        _NC_CACHE[nbufs] = _build_nc(nbufs)
    return _NC_CACHE[nbufs]


def _sel_matrices():
    # the 1/32 channel-indicator: reduce-matmuls on per-partition values
    # yield (sum over the channel's 32 partitions)/32
    sel = np.zeros((128, CBLK), dtype=np.float32)
    sel[np.arange(128), np.arange(128) % CBLK] = 1.0 / BL
    selT = np.zeros((CBLK, 128), dtype=np.float32)
    selT[np.arange(128) % CBLK, np.arange(128)] = 1.0
    return sel, selT


def pack_inputs(x, gamma, beta):
    """Full f32 inputs -> list of per-core in_maps (bf16 device layout)."""
    x16 = np.asarray(x, dtype=np.float32).astype(NP_BF16)
    gamma = np.asarray(gamma, dtype=np.float32)
    beta = np.asarray(beta, dtype=np.float32)
    # [b_hi, b_lo, core, blk, cc, hw] -> [core, blk, b_hi, b_lo, cc, hw]
    xr = np.ascontiguousarray(
        x16.reshape(BH, BL, N_CORES, N_BLOCKS, CBLK, HW).transpose(2, 3, 0, 1, 4, 5)
    )
    g = gamma.reshape(N_CORES, N_BLOCKS, CBLK)
    bt = beta.reshape(N_CORES, N_BLOCKS, CBLK)
    sel, selT = _sel_matrices()
    sel8b = sel.astype(NP_BF16)  # 1/32 is exact in bf16
    in_maps = []
    for i in range(N_CORES):
        in_maps.append(
            {
                "x": xr[i].reshape(N_HALF, 128, HW),
                "gamma": np.ascontiguousarray(g[i].T),
                "beta": np.ascontiguousarray(bt[i].T),
                "sel8b": sel8b,
                "sel8f": sel,
                "selT": selT,
            }
        )
    return in_maps


def unpack_outputs(per_core_y):
    """List of per-core y (bf16 device layout) -> full f32 (64,256,56,56)."""
    ys = np.stack(per_core_y)  # [core, blk*b_hi, 128, hw] bf16
    out = (
        ys.reshape(N_CORES, N_BLOCKS, BH, BL, CBLK, HW)
        .transpose(2, 3, 0, 1, 4, 5)
        .astype(np.float32)
        .reshape(B, C, H, W)
    )
    return out


def run(inputs, trace=False, nbufs=16):
    """Returns (full_output, BassKernelResults)."""
    nc = get_nc(nbufs)
    in_maps = pack_inputs(inputs["x"], inputs["gamma"], inputs["beta"])
    res = run_bass_kernel_spmd(nc, in_maps, list(range(N_CORES)), trace=trace)
    out = unpack_outputs([r["y"] for r in res.results])
    return out, res


def kernel(**inputs):
    out, _ = run(inputs)
    return out
